# revision 2
# baseline (speedup 1.0000x reference)
"""GAT (2-layer) Trainium2 Bass kernel — 8-core SPMD, v2.

v2 strategy (vs baseline): kill per-chunk instruction overheads.
  - Slot layout: edge j of dst-rank o in window w lives at partition o,
    chunk column j (baseline: [e%128, e//128] of e=o*d+j). Consequences:
      * er is a per-partition scalar per window -> fused into the Act-engine
        Exp bias (NO per-chunk er DMAs; baseline had 4099/layer).
      * segment-sum = strided tensor_reduce on DVE (NO S-matrix matmuls).
  - Batched gather: ONE indirect_dma_start per group of windows with a
    [128, K] int32 offset AP (baseline: 1583 per layer, ~1us SWDGE fixed
    overhead each).
  - bf16 tables: 128B rows (L1: 64 cols) / 80B rows (L2: 40 cols) halve
    gather DMA time and AllGather bytes.
  - exp(lrelu(x)) = max(exp(x), exp(0.2*x)): two fused Act Exp ops
    (bias=er resp. 0.2*er, scale=1 resp. 0.2), DVE max.
  - Tables AllGathered in segments (seg-major row numbering) so collectives
    overlap table build (L1) and layer-1 edge compute (L2).
  - Flush: normalize node-major, one PE transpose, unrotate matmul
    (feat-major out), bias per-partition, ELU; layer-2 table built directly
    from feat-major h. Final output written feat-major; host transposes.
"""
import math
import time
from contextlib import ExitStack

import numpy as np
import ml_dtypes

import concourse.bass as bass
import concourse.bacc as bacc
import concourse.mybir as mybir
import concourse.tile as tile
import concourse.bass_utils as bass_utils
import concourse.bass2jax as b2j

# ---------------------------------------------------------------- walrus flags
_orig_get_walrus_args = bass_utils.get_walrus_args


def _patched_walrus_args(*a, **kw):
    return _orig_get_walrus_args(*a, **kw) + [
        "--dge-levels=io,spill_reload,scalar_dynamic_offset,"
        "vector_dynamic_offsets,dynamic_size,transpose",
        "--dynamic-dma-scratch-size-per-partition=32768"]


bass_utils.get_walrus_args = _patched_walrus_args

P = 128
NCORES = 8
DIN = 128
DH = 64          # hidden feats (layer1 out)
DC = 40          # classes (layer2 out)
NEG = 0.2
F32 = mybir.dt.float32
BF16 = mybir.dt.bfloat16
I32 = mybir.dt.int32
GCAP = 64        # max chunks (columns) per gather group
BF = ml_dtypes.bfloat16


# ================================================================ host prep
def householder(a, n, col):
    """H (n x n) symmetric orthogonal with H @ unit(a) = e_col."""
    v = np.zeros(n, np.float64)
    v[: len(a)] = a
    na = np.linalg.norm(v)
    vu = v / na
    e = np.zeros(n, np.float64)
    e[col] = 1.0
    w = vu - e
    nw = np.linalg.norm(w)
    if nw < 1e-12:
        return np.eye(n), na
    w = w / nw
    H = np.eye(n) - 2.0 * np.outer(w, w)
    assert abs((H @ vu)[col] - 1.0) < 1e-9
    return H, na


class Plan:
    pass


def host_plan(src, dst, n_nodes):
    """Degree-sorted windows, round-robin core deal, (j,o) slot layout."""
    E = len(src)
    deg = np.bincount(dst, minlength=n_nodes)
    assert deg.max() <= GCAP, f"max degree {deg.max()} > {GCAP} unsupported"
    plan = Plan()
    plan.need_eps = bool(deg.min() == 0)
    order = np.argsort(-deg, kind="stable")  # descending degree
    per_core = math.ceil(n_nodes / NCORES)
    F = math.ceil((per_core + 1) / P)  # +1 ensures >=1 dummy rank per core
    R = F * P
    rank_of = np.full(n_nodes, -1, np.int64)
    core_of = np.full(n_nodes, -1, np.int64)
    idx = np.arange(n_nodes)
    core_of[order] = idx % NCORES
    rank_of[order] = idx // NCORES
    ranks = np.full((NCORES, R), -1, np.int64)
    ranks[core_of[order], rank_of[order]] = order
    plan.R = R
    plan.F = F
    plan.ranks = ranks
    plan.rank_of = rank_of
    plan.core_of = core_of
    plan.zero_rank = R - 1
    assert (ranks[:, plan.zero_rank] == -1).all()
    # per-window degree: window w covers global sort positions [w*1024, ...)
    windows = []
    for w in range(F):
        lo = w * P * NCORES
        dmax = int(deg[order[lo]]) if lo < n_nodes else 1
        windows.append(max(1, dmax))
    plan.windows = windows
    chunk0 = np.concatenate([[0], np.cumsum(windows)]).astype(np.int64)
    plan.chunk0 = chunk0
    plan.NCHUNK = int(chunk0[-1])

    # ---- layer segment splits (seg-major table row numbering per layer)
    def win_splits(nseg):
        bounds = [0]
        for part in np.array_split(np.arange(F), nseg):
            bounds.append(bounds[-1] + len(part))
        return [b for b in bounds]
    plan.seg1_w = win_splits(2)   # window bounds, len 3
    plan.seg2_w = win_splits(4)   # window bounds, len 5

    def gid_map(seg_w):
        """(core, rank) -> seg-major global row id, as [NCORES, R] array."""
        row_starts = [b * P for b in seg_w]  # per-core row bounds
        g = np.zeros((NCORES, R), np.int64)
        for s in range(len(seg_w) - 1):
            lo, hi = row_starts[s], row_starts[s + 1]
            n = hi - lo
            for c in range(NCORES):
                g[c, lo:hi] = NCORES * lo + c * n + np.arange(n)
        return g
    gid1 = gid_map(plan.seg1_w)
    gid2 = gid_map(plan.seg2_w)

    # ---- vectorized gidx build: slot (dst node o, edge j) -> [p=o%128,
    #      col=chunk0[w]+j] with w=o//128, per dst core.
    sidx = np.argsort(dst, kind="stable")
    dsts = dst[sidx]
    srcs = src[sidx]
    starts = np.concatenate([[0], np.cumsum(np.bincount(dsts, minlength=n_nodes))])
    j_e = np.arange(E) - starts[dsts]
    dcore = core_of[dsts]
    drank = rank_of[dsts]
    w_e = drank // P
    p_e = drank % P
    col_e = chunk0[w_e] + j_e
    score = core_of[srcs]
    srank = rank_of[srcs]
    NCH = plan.NCHUNK
    gidx1 = np.empty((NCORES, P, NCH), np.int32)
    gidx2 = np.empty((NCORES, P, NCH), np.int32)
    for c in range(NCORES):
        gidx1[c, :, :] = gid1[c, plan.zero_rank]
        gidx2[c, :, :] = gid2[c, plan.zero_rank]
    flat = (dcore * P + p_e) * NCH + col_e
    gidx1.reshape(-1)[flat] = gid1[score, srank]
    gidx2.reshape(-1)[flat] = gid2[score, srank]
    plan.gidx1 = gidx1
    plan.gidx2 = gidx2

    # ---- gather groups: consecutive windows, sum(d) <= GCAP
    groups = []  # (w_start, w_end, chunk_start, nchunks)
    w0 = 0
    while w0 < F:
        w1 = w0
        tot = 0
        while w1 < F and tot + windows[w1] <= GCAP:
            tot += windows[w1]
            w1 += 1
        assert w1 > w0
        groups.append((w0, w1, int(chunk0[w0]), tot))
        w0 = w1
    plan.groups = groups
    return plan


def host_consts(W1, al1, ar1, b1, W2, al2, ar2, b2):
    H1, na1 = householder(np.asarray(al1, np.float64), DH, DH - 1)
    H2, na2 = householder(np.asarray(al2, np.float64), DC, DC - 1)
    D1 = np.eye(DH)
    D1[DH - 1, DH - 1] = na1
    D2 = np.eye(DC)
    D2[DC - 1, DC - 1] = na2
    W1f = np.asarray(W1, np.float64)
    W2f = np.asarray(W2, np.float64)
    waug1 = np.zeros((DIN, DH + 1), np.float32)
    waug1[:, :DH] = (W1f @ H1 @ D1).astype(np.float32)
    waug1[:, DH] = (W1f @ np.asarray(ar1, np.float64)).astype(np.float32)
    waug2 = np.zeros((DH, DC + 1), np.float32)
    waug2[:, :DC] = (W2f @ H2 @ D2).astype(np.float32)
    waug2[:, DC] = (W2f @ np.asarray(ar2, np.float64)).astype(np.float32)
    un1 = (np.linalg.inv(D1) @ H1).astype(np.float32)          # [64, 64]
    un2 = (np.linalg.inv(D2) @ H2).astype(np.float32)          # [40, 40]
    ident = np.eye(P, dtype=np.float32)
    b1col = np.asarray(b1, np.float32)[:, None]                # [64, 1]
    b2col = np.asarray(b2, np.float32)[:, None]                # [40, 1]
    return dict(waug1=waug1.astype(BF), waug2=waug2.astype(BF),
                un1=un1.astype(BF), un2=un2.astype(BF),
                ident=ident.astype(BF), b1col=b1col, b2col=b2col)


# ================================================================ device build
def build_nc(plan, debug=False, ablate=()):
    R = plan.R
    F = plan.F
    NCH = plan.NCHUNK

    nc = bacc.Bacc("TRN2", target_bir_lowering=False, debug=False,
                   num_devices=NCORES, num_swdge_queues=4)
    xT = nc.declare_dram_parameter("xT", [DIN, R], BF16, isOutput=False)
    gidx1 = nc.declare_dram_parameter("gidx1", [P, NCH], I32, isOutput=False)
    gidx2 = nc.declare_dram_parameter("gidx2", [P, NCH], I32, isOutput=False)
    waug1 = nc.declare_dram_parameter("waug1", [DIN, DH + 1], BF16, isOutput=False)
    waug2 = nc.declare_dram_parameter("waug2", [DH, DC + 1], BF16, isOutput=False)
    un1 = nc.declare_dram_parameter("un1", [DH, DH], BF16, isOutput=False)
    un2 = nc.declare_dram_parameter("un2", [DC, DC], BF16, isOutput=False)
    identP = nc.declare_dram_parameter("ident", [P, P], BF16, isOutput=False)
    b1colP = nc.declare_dram_parameter("b1col", [DH, 1], F32, isOutput=False)
    b2colP = nc.declare_dram_parameter("b2col", [DC, 1], F32, isOutput=False)
    outP = nc.declare_dram_parameter("out", [DC, R], F32, isOutput=True)

    # internal DRAM
    t1_shard = nc.dram_tensor("t1_shard", [R, DH], BF16)
    t2_shard = nc.dram_tensor("t2_shard", [R, DC], BF16)
    t1_full = nc.dram_tensor("t1_full", [NCORES * R, DH], BF16,
                             addr_space="Shared")
    t2_full = nc.dram_tensor("t2_full", [NCORES * R, DC], BF16,
                             addr_space="Shared")

    rg = [list(range(NCORES))]
    Exp = mybir.ActivationFunctionType.Exp

    with ExitStack() as ctx:
        tc = ctx.enter_context(tile.TileContext(nc))
        cons = ctx.enter_context(tc.tile_pool(name="cons", bufs=1))
        big = ctx.enter_context(tc.tile_pool(name="big", bufs=1))
        sb = ctx.enter_context(tc.tile_pool(name="sb", bufs=3))
        gp = ctx.enter_context(tc.tile_pool(name="gp", bufs=3))
        rp = ctx.enter_context(tc.tile_pool(name="rp", bufs=2))
        wp = ctx.enter_context(tc.tile_pool(name="wp", bufs=2))
        fp = ctx.enter_context(tc.tile_pool(name="fp", bufs=2))
        ps = ctx.enter_context(tc.tile_pool(name="ps", bufs=2, space="PSUM"))
        pb_pool = ctx.enter_context(tc.tile_pool(name="pb", bufs=2, space="PSUM"))

        # ---- constants
        waug1_t = cons.tile([DIN, DH + 1], BF16)
        nc.sync.dma_start(out=waug1_t[:], in_=waug1[:, :])
        waug2_t = cons.tile([DH, DC + 1], BF16)
        nc.sync.dma_start(out=waug2_t[:], in_=waug2[:, :])
        un1_t = cons.tile([DH, DH], BF16)
        nc.sync.dma_start(out=un1_t[:], in_=un1[:, :])
        un2_t = cons.tile([DC, DC], BF16)
        nc.sync.dma_start(out=un2_t[:], in_=un2[:, :])
        ident_t = cons.tile([P, P], BF16)
        nc.sync.dma_start(out=ident_t[:], in_=identP[:, :])
        b1_t = cons.tile([DH, 1], F32)
        nc.sync.dma_start(out=b1_t[:], in_=b1colP[:, :])
        b2_t = cons.tile([DC, 1], F32)
        nc.sync.dma_start(out=b2_t[:], in_=b2colP[:, :])
        gidx1_t = big.tile([P, NCH], I32)
        nc.sync.dma_start(out=gidx1_t[:], in_=gidx1[:, :])
        gidx2_t = big.tile([P, NCH], I32)
        nc.sync.dma_start(out=gidx2_t[:], in_=gidx2[:, :])
        negrow = cons.tile([1, DH], BF16)
        nc.vector.memset(negrow[:], -1e30)
        negone = cons.tile([DH, 1], F32)
        nc.vector.memset(negone[:], -1.0)

        er1 = big.tile([P, F], F32)
        er1s = big.tile([P, F], F32)   # 0.2 * er1
        er2 = big.tile([P, F], F32)
        er2s = big.tile([P, F], F32)

        # ---- layer1 table build (seg-major: 2 AllGather pieces)
        xT_t = big.tile([DIN, R], BF16)
        nc.sync.dma_start(out=xT_t[:], in_=xT[:, :])
        for s in range(2):
            wlo, whi = plan.seg1_w[s], plan.seg1_w[s + 1]
            for f in range(wlo, whi):
                pb = pb_pool.tile([P, DH + 1], F32, tag="bld")
                nc.tensor.matmul(pb[:], xT_t[:, f * P:(f + 1) * P], waug1_t[:],
                                 start=True, stop=True)
                tb = sb.tile([P, DH], BF16, tag="tb")
                nc.vector.tensor_copy(tb[:], pb[:, 0:DH])
                nc.vector.tensor_copy(er1[:, f:f + 1], pb[:, DH:DH + 1])
                nc.sync.dma_start(out=t1_shard[f * P:(f + 1) * P, :], in_=tb[:])
            if s == 1:
                zr = plan.zero_rank
                nc.sync.dma_start(out=t1_shard[zr:zr + 1, :], in_=negrow[:])
            lo, hi = wlo * P, whi * P
            nc.gpsimd.collective_compute(
                "AllGather", mybir.AluOpType.bypass, replica_groups=rg,
                ins=[t1_shard[lo:hi, :].opt()],
                outs=[t1_full[NCORES * lo:NCORES * hi, :].opt()])
        nc.vector.tensor_scalar(out=er1s[:], in0=er1[:], scalar1=NEG,
                                scalar2=None, op0=mybir.AluOpType.mult)

        # ---- edge phase
        def edge_phase(layer):
            table = t1_full if layer == 1 else t2_full
            gidx_t = gidx1_t if layer == 1 else gidx2_t
            er_t = er1 if layer == 1 else er2
            ers_t = er1s if layer == 1 else er2s
            nd = DH if layer == 1 else DC     # gathered row width
            elc = nd - 1                       # el column within row
            seg2i = 0
            LAG2 = 0

            def emit_ag2(s2):
                wlo, whi = plan.seg2_w[s2], plan.seg2_w[s2 + 1]
                if wlo == whi:
                    return
                if whi == F:
                    zr = plan.zero_rank
                    nc.sync.dma_start(out=t2_shard[zr:zr + 1, 0:DC],
                                      in_=negrow[:, 0:DC])
                lo, hi = wlo * P, whi * P
                if "coll" not in ablate:
                    nc.gpsimd.collective_compute(
                        "AllGather", mybir.AluOpType.bypass,
                        replica_groups=rg,
                        ins=[t2_shard[lo:hi, :].opt()],
                        outs=[t2_full[NCORES * lo:NCORES * hi, :].opt()])
            for (w0, w1, c0, K) in plan.groups:
                gw = gp.tile([P, GCAP * nd], BF16, tag=f"gw{layer}")
                if "gather" in ablate:
                    nc.vector.memset(gw[:, :K * nd], 1.0)
                for k in range(K if "gather" not in ablate else 0):
                    inst = nc.gpsimd.indirect_dma_start(
                        out=gw[:, k * nd:(k + 1) * nd], out_offset=None,
                        in_=table[:, :],
                        in_offset=bass.IndirectOffsetOnAxis(
                            ap=gidx_t[:, c0 + k:c0 + k + 1], axis=0))
                    q = (c0 + k) % 4
                    inst.ins.queue = f"qPoolDynamic{q if q else ''}"
                g3 = gw[:].rearrange("p (k e) -> p k e", e=nd)
                cw = 0
                for w in range(w0, w1):
                    d = plan.windows[w]
                    el = g3[:, cw:cw + d, elc]
                    e1 = wp.tile([P, GCAP], BF16, tag="e1")
                    nc.scalar.activation(e1[:, :d], el, Exp,
                                         bias=er_t[:, w:w + 1], scale=1.0)
                    e2 = wp.tile([P, GCAP], BF16, tag="e2")
                    nc.scalar.activation(e2[:, :d], el, Exp,
                                         bias=ers_t[:, w:w + 1], scale=NEG)
                    wv = wp.tile([P, GCAP], BF16, tag="wv")
                    nc.vector.tensor_tensor(out=wv[:, :d], in0=e1[:, :d],
                                            in1=e2[:, :d],
                                            op=mybir.AluOpType.max)
                    sv = wp.tile([P, 1], F32, tag="sv")
                    nc.vector.tensor_reduce(sv[:], wv[:, :d],
                                            axis=mybir.AxisListType.X,
                                            op=mybir.AluOpType.add)
                    rec = wp.tile([P, 1], F32, tag="rec")
                    nc.vector.tensor_scalar(out=rec[:], in0=sv[:],
                                            scalar1=1e-30, scalar2=None,
                                            op0=mybir.AluOpType.add)
                    nc.vector.reciprocal(rec[:], rec[:])
                    r3 = rp.tile([P, GCAP * nd], BF16, tag=f"r3{layer}")
                    r3v = r3[:].rearrange("p (k e) -> p k e", e=nd)
                    wb = wv[:, :d].to_broadcast([P, d, nd])
                    if "wmult" not in ablate:
                        nc.vector.tensor_tensor(out=r3v[:, :d, :],
                                                in0=g3[:, cw:cw + d, :], in1=wb,
                                                op=mybir.AluOpType.mult)
                    hsum = fp.tile([P, nd], F32, tag="hsum")
                    rin = r3[:].rearrange("p (k e) -> p e k", e=nd)[:, :, :d]
                    if "reduce" not in ablate:
                        nc.vector.tensor_reduce(hsum[:], rin,
                                                axis=mybir.AxisListType.X,
                                                op=mybir.AluOpType.add)
                    else:
                        nc.vector.memset(hsum[:], 0.0)
                    hvn = fp.tile([P, nd], BF16, tag="hvn")
                    nc.vector.tensor_scalar(out=hvn[:], in0=hsum[:],
                                            scalar1=rec[:], scalar2=None,
                                            op0=mybir.AluOpType.mult)
                    # transpose to feat-major
                    ptr = ps.tile([nd, P], BF16, tag="tr")
                    nc.tensor.transpose(ptr[:], hvn[:], ident_t[:])
                    hT = fp.tile([nd, P], BF16, tag="hT")
                    nc.vector.tensor_copy(hT[:], ptr[:])
                    # unrotate: [feat, node]
                    un_t = un1_t if layer == 1 else un2_t
                    pun = ps.tile([nd, P], F32, tag="un")
                    nc.tensor.matmul(pun[:], un_t[:], hT[:],
                                     start=True, stop=True)
                    b_t = b1_t if layer == 1 else b2_t
                    hvB = fp.tile([nd, P], F32, tag="hvB")
                    nc.vector.tensor_scalar(out=hvB[:], in0=pun[:],
                                            scalar1=b_t[:], scalar2=None,
                                            op0=mybir.AluOpType.add)
                    if layer == 1:
                        # ELU, feat-major
                        tt = fp.tile([DH, P], F32, tag="tt")
                        nc.vector.tensor_scalar(out=tt[:], in0=hvB[:],
                                                scalar1=0.0, scalar2=None,
                                                op0=mybir.AluOpType.min)
                        nc.scalar.activation(tt[:], tt[:], Exp)
                        nc.vector.tensor_scalar(out=tt[:], in0=tt[:],
                                                scalar1=1.0, scalar2=None,
                                                op0=mybir.AluOpType.subtract)
                        hvE = fp.tile([DH, P], BF16, tag="hvE")
                        nc.vector.tensor_tensor(out=hvE[:], in0=hvB[:],
                                                in1=tt[:],
                                                op=mybir.AluOpType.max)
                        # layer2 table rows: [node, 40 | er2]
                        pt2 = ps.tile([P, DC + 1], F32, tag="t2")
                        nc.tensor.matmul(pt2[:], hvE[:], waug2_t[:],
                                         start=True, stop=True)
                        t2b = sb.tile([P, DC], BF16, tag="t2b")
                        nc.vector.tensor_copy(t2b[:], pt2[:, 0:DC])
                        nc.vector.tensor_copy(er2[:, w:w + 1],
                                              pt2[:, DC:DC + 1])
                        nc.sync.dma_start(
                            out=t2_shard[w * P:(w + 1) * P, :], in_=t2b[:])
                    else:
                        ob = fp.tile([DC, P], F32, tag="ob")
                        nc.vector.tensor_copy(ob[:], hvB[:])
                        nc.sync.dma_start(out=outP[:, w * P:(w + 1) * P],
                                          in_=ob[:])
                    cw += d
                # t2 AllGather pieces, LAG2 windows after their flushes
                if layer == 1:
                    while (seg2i < 4
                           and plan.seg2_w[seg2i + 1] + LAG2 <= w1):
                        emit_ag2(seg2i)
                        seg2i += 1
            if layer == 1:
                while seg2i < 4:
                    emit_ag2(seg2i)
                    seg2i += 1

        edge_phase(1)
        nc.vector.tensor_scalar(out=er2s[:], in0=er2[:], scalar1=NEG,
                                scalar2=None, op0=mybir.AluOpType.mult)
        edge_phase(2)

    nc.compile()
    return nc


# ================================================================ runner
class BassRunner:
    def __init__(self, nc, n_cores=NCORES):
        import jax
        from jax.experimental.shard_map import shard_map
        from jax.sharding import Mesh, PartitionSpec
        b2j.install_neuronx_cc_hook()
        self.jax = jax
        self.nc = nc
        self.n_cores = n_cores
        pname = nc.partition_id_tensor.name if nc.partition_id_tensor else None
        in_names, out_names, out_avals, zero_outs = [], [], [], []
        for alloc in nc.m.functions[0].allocations:
            if not isinstance(alloc, mybir.MemoryLocationSet):
                continue
            name = alloc.memorylocations[0].name
            if alloc.kind == "ExternalInput":
                if name != pname:
                    in_names.append(name)
            elif alloc.kind == "ExternalOutput":
                out_names.append(name)
                shape = tuple(alloc.tensor_shape)
                dtype = mybir.dt.np(alloc.dtype)
                out_avals.append(jax.core.ShapedArray(shape, dtype))
                zero_outs.append(np.zeros(shape, dtype))
        self.in_names, self.out_names = in_names, out_names
        self.out_avals, self.zero_outs = out_avals, zero_outs
        all_in = list(in_names) + list(out_names)
        if pname is not None:
            all_in.append(pname)

        def _body(*args):
            operands = list(args)
            if pname is not None:
                operands.append(b2j.partition_id_tensor())
            return tuple(b2j._bass_exec_p.bind(
                *operands, out_avals=tuple(out_avals), in_names=tuple(all_in),
                out_names=tuple(out_names), lowering_input_output_aliases=(),
                sim_require_finite=False, sim_require_nnan=False, nc=nc))

        devices = jax.devices()[:n_cores]
        self.mesh = Mesh(np.asarray(devices), ("core",))
        nio = len(in_names) + len(out_names)
        self._fn = jax.jit(
            shard_map(_body, mesh=self.mesh,
                      in_specs=(PartitionSpec("core"),) * nio,
                      out_specs=(PartitionSpec("core"),) * len(out_names),
                      check_rep=False),
            keep_unused=True)
        self._dev_in = None

    def put_inputs(self, in_maps):
        jax = self.jax
        from jax.sharding import PartitionSpec
        sharding = jax.sharding.NamedSharding(self.mesh, PartitionSpec("core"))
        concat = [np.concatenate([np.asarray(in_maps[c][n])
                                  for c in range(self.n_cores)], axis=0)
                  for n in self.in_names]
        zeros = [np.zeros((self.n_cores * z.shape[0], *z.shape[1:]), z.dtype)
                 for z in self.zero_outs]
        self._dev_in = [jax.device_put(a, sharding) for a in concat + zeros]
        jax.block_until_ready(self._dev_in)

    def run(self):
        outs = self._fn(*self._dev_in)
        self.jax.block_until_ready(outs)
        return outs

    def results(self, outs):
        res = []
        for c in range(self.n_cores):
            d = {}
            for i, name in enumerate(self.out_names):
                d[name] = np.asarray(outs[i]).reshape(
                    self.n_cores, *self.out_avals[i].shape)[c]
            res.append(d)
        return res


def make_in_maps(plan, consts, in_feat):
    in_maps = []
    feat_bf = np.asarray(in_feat, np.float32).astype(BF)
    for c in range(NCORES):
        xTc = np.zeros((DIN, plan.R), BF)
        rk = plan.ranks[c]
        valid = rk >= 0
        xTc[:, valid] = feat_bf[rk[valid]].T
        m = {"xT": xTc, "gidx1": plan.gidx1[c], "gidx2": plan.gidx2[c]}
        for k in ("waug1", "waug2", "un1", "un2", "ident", "b1col", "b2col"):
            m[k] = consts[k]
        in_maps.append(m)
    return in_maps


def unshard_out(plan, res, n_nodes):
    out_full = np.zeros((n_nodes, DC), np.float32)
    for c in range(NCORES):
        rk = plan.ranks[c]
        valid = rk >= 0
        out_full[rk[valid]] = res[c]["out"].T[valid]
    return out_full


# ================================================================ entry point
_CACHE = {}


def kernel(in_feat, src, dst, W1, al1, ar1, b1, W2, al2, ar2, b2,
           _time_out=None):
    in_feat = np.asarray(in_feat)
    src = np.asarray(src)
    dst = np.asarray(dst)
    n_nodes = in_feat.shape[0]
    key = (n_nodes, len(src), int(src[0]), int(dst[0]), int(src[-1]))
    if key not in _CACHE:
        plan = host_plan(src, dst, n_nodes)
        nc = build_nc(plan)
        runner = BassRunner(nc)
        _CACHE[key] = (plan, runner)
    else:
        plan, runner = _CACHE[key]
    consts = host_consts(W1, al1, ar1, b1, W2, al2, ar2, b2)
    runner.put_inputs(make_in_maps(plan, consts, in_feat))
    t0 = time.perf_counter()
    outs = runner.run()
    wall = time.perf_counter() - t0
    if _time_out is not None:
        _time_out.append(wall)
        _time_out.append(runner)
    res = runner.results(outs)
    return unshard_out(plan, res, n_nodes)
